# revision 1
# baseline (speedup 1.0000x reference)
"""Causal self-attention (B=4, T=2048, C=1024, H=16) on 8 trn2 NeuronCores.

Sharding: core c handles batch b = c//2 and head-group g = c%2 (8 heads).
QKV/proj weights are split column/row-wise per head-group; each core returns
a partial projection output; the host sums the two head-group partials.

Per-core pipeline (all matmuls f32r = tf32-rate, attention probs fp16):
  A) QKV^T: qkv^T tiles from w-stationary matmuls (Q,K transposed [d,t]
     layout) + V in natural [t,d] layout from xT-stationary matmuls.
  B) per head: S^T[k,q] = K^T.T @ Q^T -> ACT exp (scale=1/8, causal via
     tile skip + triangular mask) -> A^T fp16 -> AV with ones-column
     appended to V giving y^T[d,q] and the softmax denominator row ->
     normalize via reciprocal + PE partition-broadcast.
  C) out = y^T.T @ w_proj accumulated over head pairs.
"""

import sys

sys.path.insert(0, "/opt/trn_rl_repo")

import numpy as np

import concourse.bass as bass
import concourse.mybir as mybir
import concourse.tile as tile
from concourse.bass_utils import run_bass_kernel_spmd

F32 = mybir.dt.float32
F32R = mybir.dt.float32r
F16 = mybir.dt.float16
EXP = mybir.ActivationFunctionType.Exp

T = 2048
C = 1024
NHL = 8  # local heads per core
DH = 64
NT = T // 128  # 16 t/k tiles
NCT = C // 128  # 8 contraction tiles
NQ = T // 512  # 4 q chunks
NK = T // 128  # 16 k tiles


def _split_multi_waits(nc):
    """walrus on this path encodes at most ONE sem-wait per instruction;
    hoist extra waits onto same-engine no-ops inserted just before."""
    for f in nc.m.functions:
        for bb in f.blocks:
            out = []
            changed = False
            for inst in bb.instructions:
                si = inst.sync_info
                ws = list(si.on_wait) if si is not None else []
                if len(ws) > 1:
                    changed = True
                    for j, w in enumerate(ws[:-1]):
                        nop = mybir.InstNoOp(name=f"{inst.name}-wsp{j}")
                        nop.engine = inst.engine
                        nop.sync_info = mybir.SyncInfo(on_wait=[w], on_update=[])
                        out.append(nop)
                    inst.sync_info = mybir.SyncInfo(
                        on_wait=[ws[-1]], on_update=list(si.on_update)
                    )
                out.append(inst)
            if changed:
                bb.instructions = out
    return nc


def _build(opts=None):
    opts = set(opts or ())
    nc = bass.Bass(target_bir_lowering=False)
    xt_d = nc.declare_dram_parameter("xt", [C, T], F32R, isOutput=False)
    wqk_d = nc.declare_dram_parameter("wqk", [C, 1024], F32R, isOutput=False)
    wv_d = nc.declare_dram_parameter("wv", [C, 512], F32R, isOutput=False)
    wp_d = nc.declare_dram_parameter("wp", [512, C], F32R, isOutput=False)
    tri_d = nc.declare_dram_parameter("tri", [128, 128], F16, isOutput=False)
    ones_d = nc.declare_dram_parameter("ones64", [1, 64], F32R, isOutput=False)
    out_d = nc.declare_dram_parameter("out", [T, C], F32, isOutput=True)

    with tile.TileContext(nc) as tc:
        with (
            tc.tile_pool(name="qkt", bufs=1) as qkt_pool,
            tc.tile_pool(name="vsb", bufs=1) as v_pool,
            tc.tile_pool(name="ysb", bufs=1) as y_pool,
            tc.tile_pool(name="smalls", bufs=2) as small_pool,
            tc.tile_pool(name="consts", bufs=1) as const_pool,
        ):
            tri_sb = const_pool.tile([128, 128], F16, tag="tri", name="tri")
            nc.sync.dma_start(out=tri_sb[:, :], in_=tri_d.ap())
            ones_sb = const_pool.tile([1, 64], F32R, tag="ones", name="ones")
            nc.sync.dma_start(out=ones_sb[:, :], in_=ones_d.ap())

            # Q^T/K^T tiles [128(j), 2048(t)] f32r; j-tile p<4 -> Q heads
            # (2p, 2p+1); p>=4 -> K heads (2(p-4), 2(p-4)+1)
            qkt = [qkt_pool.tile([128, T], F32R, tag=f"qkt{j}", name=f"qkt{j}") for j in range(8)]
            # V tiles per k-tile: [128(t), 8*65] fp16, per-head 64 V cols + ones
            vt = [v_pool.tile([128, NHL * 65], F16, tag=f"v{k}", name=f"v{k}") for k in range(NT)]
            # y^T tiles [128(hd), 2048(t)] f32r, one per head pair
            yt = (
                []
                if "onlyA" in opts
                else [
                    y_pool.tile([128, T], F32R, tag=f"y{p}", name=f"y{p}")
                    for p in range(4)
                ]
            )

            # ---------------- Phase A: QKV projections ----------------
            with tc.tile_pool(name="xt", bufs=1) as xt_pool:
                xt = [xt_pool.tile([128, T], F32R, tag=f"xt{ci}", name=f"xt{ci}") for ci in range(NCT)]
                for ci in range(NCT):
                    for hh in range(2):  # 2 chunks/tile: fewer HWDGE launches
                        nc.sync.dma_start(
                            out=xt[ci][:, hh * 1024 : (hh + 1) * 1024],
                            in_=xt_d.ap()[
                                ci * 128 : (ci + 1) * 128, hh * 1024 : (hh + 1) * 1024
                            ],
                        )

                # Q^T / K^T: out[j(128), t] = w[c,j].T @ xT[c,t]
                # Weights loaded half-at-a-time (Q cols then K cols) as 8
                # resident [128,512] tiles each -> 16 DMA launches not 64
                # (HWDGE launch serialization dominates phase A otherwise).
                with tc.tile_pool(name="pga", bufs=2, space="PSUM") as pga_pool:
                    for half in range(2):
                        with tc.tile_pool(name=f"wqk{half}", bufs=1) as wqk_pool:
                            wts = [
                                wqk_pool.tile(
                                    [128, 512], F32R, tag=f"w{ci}", name=f"w{half}_{ci}"
                                )
                                for ci in range(NCT)
                            ]
                            for ci in range(NCT):
                                nc.sync.dma_start(
                                    out=wts[ci][:, :],
                                    in_=wqk_d.ap()[
                                        ci * 128 : (ci + 1) * 128,
                                        half * 512 : (half + 1) * 512,
                                    ],
                                )
                            for jj in range(4):
                                j = half * 4 + jj
                                pg = pga_pool.tile([128, T], F32, tag="pg", name=f"pg{j}")
                                for ci in range(1 if "qkvlite" in opts else NCT):
                                    for qc in range(NQ):
                                        nc.tensor.matmul(
                                            pg[:, qc * 512 : (qc + 1) * 512],
                                            wts[ci][:, jj * 128 : (jj + 1) * 128],
                                            xt[ci][:, qc * 512 : (qc + 1) * 512],
                                            start=(ci == 0),
                                            stop=(ci == NCT - 1) or "qkvlite" in opts,
                                        )
                                nc.vector.tensor_copy(qkt[j][:, :], pg[:, :])

                # V natural layout: out[t(128), jv(512)] = xT[c,t].T @ wv[c,jv]
                with (
                    tc.tile_pool(name="pgv", bufs=6, space="PSUM") as pgv_pool,
                    tc.tile_pool(name="wv", bufs=1) as wv_pool,
                ):
                    wvt = [
                        wv_pool.tile([128, 512], F32R, tag=f"wv{ci}", name=f"wv{ci}")
                        for ci in range(NCT)
                    ]
                    for ci in range(NCT):
                        nc.sync.dma_start(
                            out=wvt[ci][:, :],
                            in_=wv_d.ap()[ci * 128 : (ci + 1) * 128, :],
                        )
                    for tt in range(NT):
                        pv = pgv_pool.tile([128, 512], F32, tag="pv", name=f"pv{tt}")
                        for ci in range(1 if "qkvlite" in opts else NCT):
                            nc.tensor.matmul(
                                pv[:, :],
                                xt[ci][:, tt * 128 : (tt + 1) * 128],
                                wvt[ci][:, :],
                                start=(ci == 0),
                                stop=(ci == NCT - 1) or "qkvlite" in opts,
                            )
                        v3 = vt[tt].rearrange("p (l c) -> p l c", c=65)
                        nc.vector.tensor_copy(
                            v3[:, :, 0:64],
                            pv[:, :].rearrange("p (l c) -> p l c", c=64),
                        )
                        nc.vector.memset(v3[:, :, 64:65], 1.0)

            # ---------------- Phase B: attention per head ----------------
            with tc.tile_pool(name="wp", bufs=1) as wp_pool:
                wpt = [wp_pool.tile([128, C], F32R, tag=f"wp{p}", name=f"wp{p}") for p in range(4)]
                for p in range(4):
                    nc.sync.dma_start(
                        out=wpt[p][:, :], in_=wp_d.ap()[p * 128 : (p + 1) * 128, :]
                    )

                with (
                    tc.tile_pool(name="apool", bufs=2) as a_pool,
                    tc.tile_pool(name="sg", bufs=3, space="PSUM") as sg_pool,
                    tc.tile_pool(name="yq", bufs=1, space="PSUM") as yq_pool,
                    tc.tile_pool(name="rbp", bufs=1, space="PSUM") as rbp_pool,
                ):
                    for h in range(0 if "onlyA" in opts else NHL):
                        jq = h // 2
                        jk = 4 + h // 2
                        off = (h % 2) * 64
                        # -- pass 1: S^T tiles (1536-wide psum segments,
                        # double-buffered so exp overlaps next S^T), exp, mask --
                        a_tiles = []
                        for k in range(NK):
                            width = T - 128 * k
                            f = k // 4
                            at = a_pool.tile([128, width], F16, tag=f"a{k}", name=f"a{h}_{k}")
                            segs = [(f, min(f + 2, 4))]
                            if f + 2 < 4:
                                segs.append((f + 2, 4))
                            if "stlite" in opts:
                                segs = segs[:1]
                            for si, (a0, b0) in enumerate(segs):
                                sg = sg_pool.tile(
                                    [128, 1024], F32, tag="sg", name=f"sg{h}_{k}_{si}"
                                )
                                for qc in range(a0, b0):
                                    if "nost" in opts:
                                        break
                                    q0 = max(qc * 512, k * 128)
                                    q1 = (qc + 1) * 512
                                    l0 = q0 - a0 * 512
                                    nc.tensor.matmul(
                                        sg[:, l0 : l0 + (q1 - q0)],
                                        qkt[jk][off : off + 64, k * 128 : (k + 1) * 128],
                                        qkt[jq][off : off + 64, q0:q1],
                                        start=True,
                                        stop=True,
                                    )
                                gstart = max(128 * k, a0 * 512)
                                glen = b0 * 512 - gstart
                                if "noexp" in opts:
                                    nc.vector.tensor_copy(
                                        at[:, gstart - 128 * k : gstart - 128 * k + 128],
                                        sg[:, gstart - a0 * 512 : gstart - a0 * 512 + 128],
                                    )
                                else:
                                    nc.scalar.activation(
                                        at[:, gstart - 128 * k : gstart - 128 * k + glen],
                                        sg[:, gstart - a0 * 512 : gstart - a0 * 512 + glen],
                                        EXP,
                                        scale=0.125,
                                    )
                            if "nomask" not in opts:
                                nc.vector.tensor_mul(
                                    at[:, 0:128], at[:, 0:128], tri_sb[:, :]
                                )
                            a_tiles.append(at)
                        # -- pass 2: AV + denominator + normalize --
                        for qc in range(NQ if "noav" not in opts else 0):
                            yq = yq_pool.tile([65, 512], F32, tag="yq", name=f"yq{h}_{qc}")
                            klast = 0 if "avlite" in opts else (4 * qc + 3)
                            for k in range(klast + 1):
                                vsl = vt[k][:, h * 65 : (h + 1) * 65]
                                if k >= 4 * qc:  # diagonal tile
                                    n = 512 - (128 * k - 512 * qc)
                                    nc.tensor.matmul(
                                        yq[:, 512 - n : 512],
                                        vsl,
                                        a_tiles[k][:, 0:n],
                                        start=(k == 0),
                                        stop=(k == klast),
                                    )
                                else:
                                    c0 = qc * 512 - 128 * k
                                    nc.tensor.matmul(
                                        yq[:, :],
                                        vsl,
                                        a_tiles[k][:, c0 : c0 + 512],
                                        start=(k == 0),
                                        stop=(k == klast),
                                    )
                            if "nonorm" in opts:
                                nc.vector.tensor_copy(
                                    yt[jq][off : off + 64, qc * 512 : (qc + 1) * 512],
                                    yq[0:64, :],
                                )
                            else:
                                r = small_pool.tile([1, 512], F32R, tag="recip", name=f"r{h}_{qc}")
                                with nc.allow_low_precision(reason="f32r is fp32-width"):
                                    nc.vector.reciprocal(r[:, :], yq[64:65, :])
                                rbp = rbp_pool.tile([64, 512], F32, tag="rbp", name=f"rbp{h}_{qc}")
                                nc.tensor.matmul(
                                    rbp[:, :], ones_sb[:, :], r[:, :], start=True, stop=True
                                )
                                rb = small_pool.tile([64, 512], F32, tag="rb", name=f"rb{h}_{qc}")
                                nc.vector.tensor_copy(rb[:, :], rbp[:, :])
                                nc.vector.tensor_mul(
                                    yt[jq][off : off + 64, qc * 512 : (qc + 1) * 512],
                                    yq[0:64, :],
                                    rb[:, :],
                                )

                # ---------------- Phase C: output projection ----------------
                with (
                    tc.tile_pool(name="pj", bufs=6, space="PSUM") as pj_pool,
                    tc.tile_pool(name="ost", bufs=6) as ost_pool,
                ):
                    for tt in range(0 if ("onlyA" in opts or "noC" in opts) else NT):
                        for jc in range(2):
                            pj = pj_pool.tile([128, 512], F32, tag="pj", name=f"pj{tt}_{jc}")
                            for p in range(4):
                                nc.tensor.matmul(
                                    pj[:, :],
                                    yt[p][:, tt * 128 : (tt + 1) * 128],
                                    wpt[p][:, jc * 512 : (jc + 1) * 512],
                                    start=(p == 0),
                                    stop=(p == 3),
                                )
                            ot = ost_pool.tile([128, 512], F32, tag="ost", name=f"ost{tt}_{jc}")
                            nc.scalar.copy(ot[:, :], pj[:, :])
                            nc.sync.dma_start(
                                out=out_d.ap()[
                                    tt * 128 : (tt + 1) * 128,
                                    jc * 512 : (jc + 1) * 512,
                                ],
                                in_=ot[:, :],
                            )

    _split_multi_waits(nc)
    return nc


_CACHED = {}


def _get_program():
    if "nc" not in _CACHED:
        _CACHED["nc"] = _build()
    return _CACHED["nc"]


def _shard_inputs(x, w_qkv, w_proj):
    x = np.ascontiguousarray(x, dtype=np.float32)
    w_qkv = np.ascontiguousarray(w_qkv, dtype=np.float32)
    w_proj = np.ascontiguousarray(w_proj, dtype=np.float32)
    tri = np.triu(np.ones((128, 128), dtype=np.float16))
    ones64 = np.ones((1, 64), dtype=np.float32)
    in_maps = []
    for core in range(8):
        b, g = core // 2, core % 2
        xt = np.ascontiguousarray(x[b].T)
        wqk = np.ascontiguousarray(
            np.concatenate(
                [
                    w_qkv[:, g * 512 : g * 512 + 512],
                    w_qkv[:, 1024 + g * 512 : 1024 + g * 512 + 512],
                ],
                axis=1,
            )
        )
        wv = np.ascontiguousarray(w_qkv[:, 2048 + g * 512 : 2048 + g * 512 + 512])
        wp = np.ascontiguousarray(w_proj[g * 512 : (g + 1) * 512, :])
        in_maps.append(
            {"xt": xt, "wqk": wqk, "wv": wv, "wp": wp, "tri": tri, "ones64": ones64}
        )
    return in_maps


def kernel(x, w_qkv, w_proj, _trace=False, _result_box=None):
    nc = _get_program()
    in_maps = _shard_inputs(x, w_qkv, w_proj)
    res = run_bass_kernel_spmd(nc, in_maps, list(range(8)), trace=_trace)
    if _result_box is not None:
        _result_box.append(res)
    B = x.shape[0]
    out = np.empty((B, T, C), dtype=np.float32)
    for b in range(B):
        out[b] = res.results[2 * b]["out"] + res.results[2 * b + 1]["out"]
    return out



# revision 15
# speedup vs baseline: 1.1870x; 1.1870x over previous
"""Causal self-attention (B=4, T=2048, C=1024, H=16) on 8 trn2 NeuronCores.

Sharding: core c handles batch b = c//2 and head-group g = c%2 (8 heads).
QKV/proj weights are split column/row-wise per head-group; each core returns
a partial projection output (fp16); the host sums the two head-group partials.

Per-core pipeline (all attention math fp16; QKV inputs fp8 hi/lo):
  A) QKV: host ships x^T and the x32-scaled weights as fp8 (hi, lo-residual)
     pairs; Q^T/K^T/V computed as 3-product DoubleRow matmuls
     (xh*wh + xh*wl + xl*wh, 256-deep contraction per pass) -> fp16.
     The x32 weight scaling keeps the lo residuals above fp8's subnormal
     floor; the scale is folded into exp (Q,K) and the normalize (V).
  B) per head: S^T[k,q] = K^T.T @ Q^T (fp16) -> ACT exp(scale=1/(8*32*32))
     -> A^T fp16 packed-causal -> diag tri mask (DVE) -> AV per q-tile:
     out[128q, 64d+denom] accumulating over k-tiles (V carries a ones
     column) -> DVE per-partition normalize (recip + tensor_scalar mul,
     second scalar de-scales by 1/32) -> y fp16.
  B5) PE transposes y[q,d] -> y^T[d,q] fp16 via identity matmuls.
  C) out = y^T.T @ w_proj (fp16) accumulated over head pairs.

Emission is software-pipelined: QKV chunks, AV, and transpose units are
interleaved between S^T k-tiles so the ACT exp stream stays fed.
"""

import sys

sys.path.insert(0, "/opt/trn_rl_repo")

import numpy as np
import ml_dtypes

import concourse.bass as bass
import concourse.mybir as mybir
import concourse.tile as tile
from concourse.bass_utils import run_bass_kernel_spmd

F32 = mybir.dt.float32
F16 = mybir.dt.float16
F8 = mybir.dt.float8e4
DR = mybir.MatmulPerfMode.DoubleRow
EXP = mybir.ActivationFunctionType.Exp

T = 2048
C = 1024
NHL = 8  # local heads per core
NCT = C // 128  # 8 contraction tiles
NT = T // 128  # 16 t/k tiles
WSC = 32.0  # host-side weight scale (keeps fp8 lo-residuals normal)

# A^T packed-causal layout: k-tile k spans q in [128k, 2048), width 2048-128k.
SLOT = []
_o = 0
for _k in range(NT):
    SLOT.append(_o)
    _o += T - 128 * _k
A_COLS = _o  # 17408


def _split_multi_waits(nc):
    """walrus encodes at most ONE sem-wait per instruction; hoist extra
    waits onto same-engine no-ops inserted just before."""
    for f in nc.m.functions:
        for bb in f.blocks:
            out = []
            changed = False
            for inst in bb.instructions:
                si = inst.sync_info
                ws = list(si.on_wait) if si is not None else []
                if len(ws) > 1:
                    changed = True
                    for j, w in enumerate(ws[:-1]):
                        nop = mybir.InstNoOp(name=f"{inst.name}-wsp{j}")
                        nop.engine = inst.engine
                        nop.sync_info = mybir.SyncInfo(on_wait=[w], on_update=[])
                        out.append(nop)
                    inst.sync_info = mybir.SyncInfo(
                        on_wait=[ws[-1]], on_update=list(si.on_update)
                    )
                out.append(inst)
            if changed:
                bb.instructions = out
    return nc


def _build():
    nc = bass.Bass(target_bir_lowering=True)
    xh_d = nc.declare_dram_parameter("xh", [C, T], F8, isOutput=False)
    xl_d = nc.declare_dram_parameter("xl", [C, T], F8, isOutput=False)
    wqkh_d = nc.declare_dram_parameter("wqkh", [C, 1024], F8, isOutput=False)
    wqkl_d = nc.declare_dram_parameter("wqkl", [C, 1024], F8, isOutput=False)
    wvh_d = nc.declare_dram_parameter("wvh", [C, 512], F8, isOutput=False)
    wvl_d = nc.declare_dram_parameter("wvl", [C, 512], F8, isOutput=False)
    wp_d = nc.declare_dram_parameter("wp", [512, C], F16, isOutput=False)
    tri_d = nc.declare_dram_parameter("tri", [128, 128], F16, isOutput=False)
    id_d = nc.declare_dram_parameter("ident", [128, 128], F16, isOutput=False)
    out_d = nc.declare_dram_parameter("out", [T, C], F16, isOutput=True)

    with tile.TileContext(nc) as tc:
        with (
            tc.tile_pool(name="xin", bufs=1) as x_pool,
            tc.tile_pool(name="win", bufs=1) as w_pool,
            tc.tile_pool(name="qkt", bufs=4) as qkt_pool,
            tc.tile_pool(name="vsb", bufs=1) as v_pool,
            tc.tile_pool(name="ah", bufs=2) as a_pool,
            tc.tile_pool(name="ysb", bufs=2) as ysb_pool,
            tc.tile_pool(name="ynorm", bufs=1) as yn_pool,
            tc.tile_pool(name="ytp", bufs=1) as yt_pool,
            tc.tile_pool(name="consts", bufs=1) as const_pool,
        ):
            # ---- input DMAs (few big launches; they serialize on the DMA
            # device in issue order, so most-urgent first) ----
            wqkh = w_pool.tile([128, NCT * 1024], F8, tag="wqkh", name="wqkh")
            nc.sync.dma_start(
                out=wqkh.rearrange("p (c j) -> p c j", c=NCT)[:, :, :],
                in_=wqkh_d.ap().rearrange("(c p) j -> p c j", p=128),
            )
            xh = x_pool.tile([128, NCT * T], F8, tag="xh", name="xh")
            xl = x_pool.tile([128, NCT * T], F8, tag="xl", name="xl")
            for half in range(2):
                nc.sync.dma_start(
                    out=xh.rearrange("p (c t) -> p c t", c=NCT)[
                        :, half * 4 : half * 4 + 4, :
                    ],
                    in_=xh_d.ap()[half * 512 : (half + 1) * 512, :].rearrange(
                        "(c p) t -> p c t", p=128
                    ),
                )
            for half in range(2):
                nc.sync.dma_start(
                    out=xl.rearrange("p (c t) -> p c t", c=NCT)[
                        :, half * 4 : half * 4 + 4, :
                    ],
                    in_=xl_d.ap()[half * 512 : (half + 1) * 512, :].rearrange(
                        "(c p) t -> p c t", p=128
                    ),
                )
            wqkl = w_pool.tile([128, NCT * 1024], F8, tag="wqkl", name="wqkl")
            nc.sync.dma_start(
                out=wqkl.rearrange("p (c j) -> p c j", c=NCT)[:, :, :],
                in_=wqkl_d.ap().rearrange("(c p) j -> p c j", p=128),
            )
            wvh = w_pool.tile([128, NCT * 512], F8, tag="wvh", name="wvh")
            nc.sync.dma_start(
                out=wvh.rearrange("p (c j) -> p c j", c=NCT)[:, :, :],
                in_=wvh_d.ap().rearrange("(c p) j -> p c j", p=128),
            )
            wvl = w_pool.tile([128, NCT * 512], F8, tag="wvl", name="wvl")
            nc.sync.dma_start(
                out=wvl.rearrange("p (c j) -> p c j", c=NCT)[:, :, :],
                in_=wvl_d.ap().rearrange("(c p) j -> p c j", p=128),
            )
            tri = const_pool.tile([128, 128], F16, tag="tri", name="tri")
            nc.sync.dma_start(out=tri[:, :], in_=tri_d.ap())
            ident = const_pool.tile([128, 128], F16, tag="ident", name="ident")
            nc.sync.dma_start(out=ident[:, :], in_=id_d.ap())
            wp = w_pool.tile([128, 4 * 1024], F16, tag="wp", name="wp")
            nc.sync.dma_start(
                out=wp.rearrange("p (c j) -> p c j", c=4)[:, :, :],
                in_=wp_d.ap().rearrange("(c p) j -> p c j", p=128),
            )

            # 3-dim views for DoubleRow pair slicing
            xh3 = xh.rearrange("p (c t) -> p c t", c=NCT)
            xl3 = xl.rearrange("p (c t) -> p c t", c=NCT)
            wqkh3 = wqkh.rearrange("p (c j) -> p c j", c=NCT)
            wqkl3 = wqkl.rearrange("p (c j) -> p c j", c=NCT)
            wvh3 = wvh.rearrange("p (c j) -> p c j", c=NCT)
            wvl3 = wvl.rearrange("p (c j) -> p c j", c=NCT)

            # persistent sbuf tensors; qkt is a 4-slot ring reused j0,j4,j1,
            # j5 -> j2,j6,j3,j7 (slots freed once both reader heads are done)
            qkt = {}
            v_all = v_pool.tile([128, NHL * NT * 65], F16, tag="vall", name="v_all")
            v4 = v_all.rearrange("p (h k c) -> p h k c", h=NHL, c=65)
            ynorm = yn_pool.tile([128, NT * 512], F16, tag="yn", name="ynorm")
            yt = [
                yt_pool.tile([128, T], F16, tag=f"yt{p}", name=f"yt{p}")
                for p in range(4)
            ]

            a_heads = {}

            with (
                tc.tile_pool(name="yb", bufs=2, space="PSUM") as yb_pool,
                tc.tile_pool(name="pt", bufs=1, space="PSUM") as pt_pool,
                tc.tile_pool(name="sg", bufs=2, space="PSUM") as sg_pool,
                tc.tile_pool(name="pga", bufs=1, space="PSUM") as pga_pool,
            ):

                def make_qk_unit(jt, ch):
                    """Q^T/K^T 1024-col chunk via 3-product DR on the sg ring."""

                    def emit():
                        if jt not in qkt:
                            qkt[jt] = qkt_pool.tile(
                                [128, T], F16, tag="qkt", name=f"qkt{jt}"
                            )
                        pg = sg_pool.tile([128, 1024], F32, tag="sg", name=f"pg{jt}_{ch}")
                        for sub in range(2):
                            t0 = ch * 1024 + sub * 512
                            n_mm = 0
                            for wsb, xsb in ((wqkh3, xh3), (wqkh3, xl3), (wqkl3, xh3)):
                                for cp in range(NCT // 2):
                                    n_mm += 1
                                    nc.tensor.matmul(
                                        pg[:, sub * 512 : (sub + 1) * 512],
                                        wsb[:, 2 * cp : 2 * cp + 2, jt * 128 : (jt + 1) * 128],
                                        xsb[:, 2 * cp : 2 * cp + 2, t0 : t0 + 512],
                                        start=(n_mm == 1),
                                        stop=(n_mm == 12),
                                        perf_mode=DR,
                                    )
                        nc.vector.tensor_copy(
                            qkt[jt][:, ch * 1024 : (ch + 1) * 1024], pg[:, :]
                        )

                    return emit

                def make_v_unit(tt):
                    """V t-tile via 3-product DR; out [128 t, 512 jv] fp16."""

                    def emit():
                        pg = pga_pool.tile([128, 512], F32, tag="pga", name=f"pv{tt}")
                        n_mm = 0
                        for wsb, xsb in ((wvh3, xh3), (wvh3, xl3), (wvl3, xh3)):
                            for cp in range(NCT // 2):
                                n_mm += 1
                                nc.tensor.matmul(
                                    pg[:, :],
                                    xsb[:, 2 * cp : 2 * cp + 2, tt * 128 : (tt + 1) * 128],
                                    wsb[:, 2 * cp : 2 * cp + 2, :],
                                    start=(n_mm == 1),
                                    stop=(n_mm == 12),
                                    perf_mode=DR,
                                )
                        nc.vector.tensor_copy(
                            v4[:, :, tt, 0:64],
                            pg[:, :].rearrange("p (h c) -> p h c", c=64),
                        )

                    return emit

                def emit_S_k(h, k):
                    jq, jk = h // 2, 4 + h // 2
                    off = (h % 2) * 64
                    ah = a_heads[h]
                    base = SLOT[k] - 128 * k  # col for abs q: base + q
                    f = k // 4
                    segs = [(f, min(f + 2, 4))]
                    if f + 2 < 4:
                        segs.append((f + 2, 4))
                    for si, (a0, b0) in enumerate(segs):
                        sg = sg_pool.tile(
                            [128, 1024], F32, tag="sg", name=f"sg{h}_{k}_{si}"
                        )
                        for qc in range(a0, b0):
                            q0 = max(qc * 512, k * 128)
                            q1 = (qc + 1) * 512
                            nc.tensor.matmul(
                                sg[:, q0 - a0 * 512 : q1 - a0 * 512],
                                qkt[jk][off : off + 64, k * 128 : (k + 1) * 128],
                                qkt[jq][off : off + 64, q0:q1],
                                start=True,
                                stop=True,
                            )
                        gstart = max(128 * k, a0 * 512)
                        glen = b0 * 512 - gstart
                        nc.scalar.activation(
                            ah[:, base + gstart : base + gstart + glen],
                            sg[:, gstart - a0 * 512 : gstart - a0 * 512 + glen],
                            EXP,
                            scale=0.125 / (WSC * WSC),
                        )
                        if si == 0:
                            d0 = SLOT[k]
                            nc.vector.tensor_mul(
                                ah[:, d0 : d0 + 128], ah[:, d0 : d0 + 128], tri[:, :]
                            )

                def make_av_unit(h, b2, qts):
                    def emit():
                        ah = a_heads[h]
                        yb = yb_pool.tile([128, 512], F32, tag="yb", name=f"yb{h}_{b2}")
                        for j, qt in enumerate(qts):
                            for k in range(qt + 1):
                                nc.tensor.matmul(
                                    yb[:, 65 * j : 65 * j + 65],
                                    ah[
                                        :,
                                        SLOT[k] + 128 * (qt - k) : SLOT[k] + 128 * (qt - k) + 128,
                                    ],
                                    v4[:, h, k, :],
                                    start=(k == 0),
                                    stop=(k == qt),
                                )
                        nb = len(qts)
                        rec = ysb_pool.tile([128, 8], F32, tag="rec", name=f"rec{h}_{b2}")
                        with nc.allow_low_precision(reason="f32 recip of f32"):
                            nc.vector.reciprocal(rec[:, 0:nb], yb[:, 64 : 65 * nb : 65])
                        for j, qt in enumerate(qts):
                            nc.vector.tensor_scalar(
                                ynorm[:, qt * 512 + h * 64 : qt * 512 + h * 64 + 64],
                                yb[:, 65 * j : 65 * j + 64],
                                rec[:, j : j + 1],
                                1.0 / WSC,
                                mybir.AluOpType.mult,
                                mybir.AluOpType.mult,
                            )

                    return emit

                def make_b5_unit(h, quarter):
                    def emit():
                        off = (h % 2) * 64
                        pt = pt_pool.tile(
                            [64, 512], F16, tag="pt", name=f"pt{h}_{quarter}"
                        )
                        for jj in range(4):
                            qt = quarter * 4 + jj
                            nc.tensor.transpose(
                                pt[:, jj * 128 : (jj + 1) * 128],
                                ynorm[:, qt * 512 + h * 64 : qt * 512 + h * 64 + 64],
                                ident[:, :],
                            )
                        nc.vector.tensor_copy(
                            yt[h // 2][off : off + 64, quarter * 512 : (quarter + 1) * 512],
                            pt[:, :],
                        )

                    return emit

                def av_units(h):
                    return [
                        make_av_unit(h, b2, qts)
                        for b2, qts in enumerate(
                            ([0, 1, 2, 3, 4, 5, 6], [7, 8, 9, 10, 11, 12, 13], [14, 15])
                        )
                    ]

                def b5_units(h):
                    return [make_b5_unit(h, q) for q in range(4)]

                def ones_unit():
                    def emit():
                        nc.vector.memset(v4[:, :, :, 64:65], 1.0)

                    return emit

                # prologue: the minimum head 0 needs: all of j0, K cols 0:1024
                make_qk_unit(4, 0)()
                make_qk_unit(0, 0)()
                make_qk_unit(0, 1)()

                fillers = {
                    0: [make_qk_unit(4, 1)]
                    + [make_qk_unit(jt, ch) for jt in (1, 5) for ch in range(2)]
                    + [make_v_unit(tt) for tt in range(4)],
                    1: [make_v_unit(tt) for tt in range(4, 16)]
                    + [ones_unit()]
                    + av_units(0),
                    2: av_units(1)
                    + [make_qk_unit(jt, ch) for jt in (2, 6) for ch in range(2)],
                    3: av_units(2) + b5_units(0),
                    4: av_units(3)
                    + [make_qk_unit(3, ch) for ch in range(2)]
                    + b5_units(1),
                    5: av_units(4)
                    + [make_qk_unit(7, ch) for ch in range(2)]
                    + b5_units(2),
                    6: av_units(5) + b5_units(3) + b5_units(4),
                    7: av_units(6) + b5_units(5) + b5_units(6),
                }
                for h in range(NHL):
                    a_heads[h] = a_pool.tile([128, A_COLS], F16, tag="ah", name=f"a{h}")
                    fl = fillers[h]
                    done = 0
                    for k in range(NT):
                        emit_S_k(h, k)
                        want = (k + 1) * len(fl) // NT
                        while done < want:
                            fl[done]()
                            done += 1
                for u in av_units(7) + b5_units(7):
                    u()

            # ---- C: output projection (fp16) ----
            with (
                tc.tile_pool(name="pj", bufs=6, space="PSUM") as pj_pool,
                tc.tile_pool(name="ost", bufs=2) as ost_pool,
            ):
                for tt in range(NT):
                    ot = ost_pool.tile([128, 1024], F16, tag="ost", name=f"ost{tt}")
                    for jc in range(2):
                        pj = pj_pool.tile([128, 512], F32, tag="pj", name=f"pj{tt}_{jc}")
                        for p in range(4):
                            nc.tensor.matmul(
                                pj[:, :],
                                yt[p][:, tt * 128 : (tt + 1) * 128],
                                wp[:, p * 1024 + jc * 512 : p * 1024 + (jc + 1) * 512],
                                start=(p == 0),
                                stop=(p == 3),
                            )
                        if tt % 2 == 0:
                            nc.scalar.copy(ot[:, jc * 512 : (jc + 1) * 512], pj[:, :])
                        else:
                            nc.vector.tensor_copy(
                                ot[:, jc * 512 : (jc + 1) * 512], pj[:, :]
                            )
                    nc.sync.dma_start(
                        out=out_d.ap()[tt * 128 : (tt + 1) * 128, :], in_=ot[:, :]
                    )

    return nc


_CACHED = {}


def _get_program():
    if "nc" not in _CACHED:
        _CACHED["nc"] = _split_multi_waits(_build())
    return _CACHED["nc"]


def _get_program_nosplit():
    if "nc_ns" not in _CACHED:
        _CACHED["nc_ns"] = _build()
    return _CACHED["nc_ns"]


def _q8(a):
    return np.clip(a, -240.0, 240.0).astype(ml_dtypes.float8_e4m3)


def _shard_inputs(x, w_qkv, w_proj):
    x = np.ascontiguousarray(x, dtype=np.float32)
    w_qkv = np.ascontiguousarray(w_qkv, dtype=np.float32)
    w_proj = np.ascontiguousarray(w_proj, dtype=np.float32)
    tri = np.triu(np.ones((128, 128), dtype=np.float32)).astype(np.float16)
    ident = np.eye(128, dtype=np.float16)
    in_maps = []
    for core in range(8):
        b, g = core // 2, core % 2
        xt = np.ascontiguousarray(x[b].T)
        xh = _q8(xt)
        xl = _q8(xt - xh.astype(np.float32))
        wqk = (
            np.concatenate(
                [
                    w_qkv[:, g * 512 : g * 512 + 512],
                    w_qkv[:, 1024 + g * 512 : 1024 + g * 512 + 512],
                ],
                axis=1,
            )
            * WSC
        )
        wqkh = _q8(wqk)
        wqkl = _q8(wqk - wqkh.astype(np.float32))
        wv = w_qkv[:, 2048 + g * 512 : 2048 + g * 512 + 512] * WSC
        wvh = _q8(wv)
        wvl = _q8(wv - wvh.astype(np.float32))
        wp = np.ascontiguousarray(w_proj[g * 512 : (g + 1) * 512, :]).astype(
            np.float16
        )
        in_maps.append(
            {
                "xh": xh,
                "xl": xl,
                "wqkh": wqkh,
                "wqkl": wqkl,
                "wvh": wvh,
                "wvl": wvl,
                "wp": wp,
                "tri": tri,
                "ident": ident,
            }
        )
    return in_maps


def kernel(x, w_qkv, w_proj, _trace=False, _result_box=None):
    nc = _get_program()
    in_maps = _shard_inputs(x, w_qkv, w_proj)
    res = run_bass_kernel_spmd(nc, in_maps, list(range(8)), trace=_trace)
    if _result_box is not None:
        _result_box.append(res)
    B = x.shape[0]
    out = np.empty((B, T, C), dtype=np.float32)
    for b in range(B):
        out[b] = res.results[2 * b]["out"].astype(np.float32) + res.results[
            2 * b + 1
        ]["out"].astype(np.float32)
    return out
